# revision 23
# baseline (speedup 1.0000x reference)
# Trainium2 Bass kernel for MergedColumnParallelLinearWithTopping
# (base column-parallel GEMM + per-token LoRA "topping", Punica-style).
#
# Math per core c (of 8, column-parallel over the 2*BDIM output dim):
#   out_c = x @ Wc.T + ((x @ Ac) * Mc) @ Bc
# where Ac = concat_l A_buffer[l,:,half]  [D, L*R]
#       Bc = concat_l B_buffer[l,:,cols]  [L*R, CPC]
#       M[t, l*R+r] = (weight_indices[t] == l)   (host-precomputed one-hot)
# This turns the per-token gather into two dense GEMMs that accumulate in
# the same PSUM group as the base GEMM.  All matmuls run as float32r
# (full PE rate for moving free dim >= 256).
#
# Self-contained: hardcodes shapes, builds the Bass program, shards inputs,
# runs on cores 0-7 via run_bass_kernel_spmd, reassembles the full output.

import numpy as np

# Problem shapes (hardcoded per spec)
T, D = 2048, 2048
L, R = 16, 16
BDIM = 5632
NCORES = 8
CPC = 2 * BDIM // NCORES  # 1408 output cols per core
P = 128
KO = D // P               # 16 contraction chunks
TS = 512                  # token-slice (matmul moving free dim)
NT = T // TS              # 4
MCH = CPC // P            # 11 output-column chunks per core
LR = L * R                # 256 (one half's lora rows)
LRO = LR // P             # 2

_PROGRAM_CACHE = {}


def _build_program():
    import concourse.bacc as bacc
    import concourse.tile as tile
    from concourse import mybir

    f32 = mybir.dt.float32
    f32r = mybir.dt.float32r
    bf16 = mybir.dt.bfloat16

    nc = bacc.Bacc("TRN2", target_bir_lowering=False, debug=False)

    xt = nc.dram_tensor("xt", [D, T], f32r, kind="ExternalInput").ap()
    wt = nc.dram_tensor("wt", [D, CPC], f32r, kind="ExternalInput").ap()
    ac = nc.dram_tensor("ac", [D, LR], f32r, kind="ExternalInput").ap()
    bc = nc.dram_tensor("bc", [LR, CPC], f32r, kind="ExternalInput").ap()
    mt = nc.dram_tensor("mt", [LR, T], bf16, kind="ExternalInput").ap()
    out = nc.dram_tensor("out", [CPC, T], f32, kind="ExternalOutput").ap()

    xt_r = xt.rearrange("(ko p) t -> p ko t", p=P)
    wt_r = wt.rearrange("(ko p) n -> p ko n", p=P)
    ac_r = ac.rearrange("(ko p) c -> p ko c", p=P)
    bc_r = bc.rearrange("(o p) n -> p o n", p=P)
    mt_r = mt.rearrange("(o p) t -> p o t", p=P)
    out_r = out.rearrange("(mo p) t -> p mo t", p=P)

    with tile.TileContext(nc) as tc:
        with (
            tc.tile_pool(name="xres", bufs=NT) as xpool,
            tc.tile_pool(name="wpool", bufs=2) as wpool,
            tc.tile_pool(name="consts", bufs=1) as cpool,
            tc.tile_pool(name="mtp", bufs=NT) as mtpool,
            tc.tile_pool(name="outp", bufs=3) as outpool,
            tc.tile_pool(name="psout", bufs=4, space="PSUM") as psout,
            tc.tile_pool(name="psxa", bufs=2, space="PSUM") as psxa,
        ):
            # Split DMAs that feed matmul operands into k-groups: the fused
            # fp32r matmul's LDWEIGHTS has very few semaphore-wait slots, so
            # each matmul must depend on at most one small DMA.
            KG = 4  # k-chunks per sub-DMA

            # Constants resident in SBUF
            a_sb = cpool.tile([P, KO, LR], f32r, name="a_sb")
            for kg in range(0, KO, KG):
                nc.sync.dma_start(
                    a_sb[:, kg:kg + KG, :], ac_r[:, kg:kg + KG, :]
                )
            b_sb = cpool.tile([P, LRO, CPC], f32r, name="b_sb")
            for o in range(LRO):
                nc.sync.dma_start(b_sb[:, o, :], bc_r[:, o, :])
            # masked x@A activation, filled per token-slice below
            xam = cpool.tile([P, LRO, T], f32r, name="xam")

            # x fully resident, loaded as NT independent slices so deps are
            # per-slice (and per k-group within a slice)
            x_sb = []
            for t in range(NT):
                xs = xpool.tile([P, KO, TS], f32r, name=f"x{t}", tag="x")
                for kg in range(0, KO, KG):
                    nc.sync.dma_start(
                        xs[:, kg:kg + KG, :],
                        xt_r[:, kg:kg + KG, t * TS:(t + 1) * TS],
                    )
                x_sb.append(xs)

            def w_load(m):
                # single DMA per chunk: one queue, so slot-reuse WAW is one
                # semaphore and the guard absorbs the single data wait
                wtile = wpool.tile([P, KO, P], f32r, name=f"w{m}", tag="w")
                nc.sync.dma_start(wtile[:], wt_r[:, :, m * P:(m + 1) * P])
                return wtile

            w_tiles = {0: w_load(0)}

            def base_group(m, wtile, t):
                # one [128, TS] output tile: 16 base matmuls + 2 lora matmuls
                # accumulating in the same PSUM bank
                ps = psout.tile([P, TS], f32, name=f"ps_{m}_{t}", tag="ps")
                for k in range(KO):
                    nc.tensor.matmul(
                        ps[:],
                        lhsT=wtile[:, k, :],
                        rhs=x_sb[t][:, k, :],
                        start=(k == 0),
                        stop=False,
                    )
                for k2 in range(LRO):
                    nc.tensor.matmul(
                        ps[:],
                        lhsT=b_sb[:, k2, m * P:(m + 1) * P],
                        rhs=xam[:, k2, t * TS:(t + 1) * TS],
                        start=False,
                        stop=(k2 == LRO - 1),
                    )
                o = outpool.tile([P, TS], f32, name=f"o_{m}_{t}", tag="o")
                nc.any.tensor_copy(out=o[:], in_=ps[:])
                nc.sync.dma_start(out_r[:, m, t * TS:(t + 1) * TS], o[:])

            # Pass 1 over token-slices: compute masked XA, then first W chunk
            for t in range(NT):
                mt_sb = mtpool.tile([P, LRO, TS], bf16, name=f"mt{t}", tag="mt")
                for o in range(LRO):
                    nc.sync.dma_start(
                        mt_sb[:, o, :], mt_r[:, o, t * TS:(t + 1) * TS]
                    )
                for mp in range(LRO):
                    pxa = psxa.tile([P, TS], f32, name=f"pxa_{t}_{mp}", tag="pxa")
                    for k in range(KO):
                        nc.tensor.matmul(
                            pxa[:],
                            lhsT=a_sb[:, k, mp * P:(mp + 1) * P],
                            rhs=x_sb[t][:, k, :],
                            start=(k == 0),
                            stop=(k == KO - 1),
                        )
                    nc.vector.tensor_tensor(
                        xam[:, mp, t * TS:(t + 1) * TS],
                        pxa[:],
                        mt_sb[:, mp, :],
                        mybir.AluOpType.mult,
                    )
                if t == 0:
                    w_tiles[1] = w_load(1)
                base_group(0, w_tiles[0], t)

            # Remaining W chunks, x stays resident
            for m in range(1, MCH):
                if m + 1 < MCH and (m + 1) not in w_tiles:
                    w_tiles[m + 1] = w_load(m + 1)
                for t in range(NT):
                    base_group(m, w_tiles[m], t)

    nc.compile()
    return nc


def get_program():
    if "nc" not in _PROGRAM_CACHE:
        _PROGRAM_CACHE["nc"] = _build_program()
    return _PROGRAM_CACHE["nc"]


def make_in_maps(x, W, A_buffer, B_buffer, weight_indices):
    x = np.ascontiguousarray(np.asarray(x, dtype=np.float32))
    W = np.asarray(W, dtype=np.float32)
    A = np.asarray(A_buffer, dtype=np.float32)
    B = np.asarray(B_buffer, dtype=np.float32)
    wi = np.asarray(weight_indices).astype(np.int64)

    xt = np.ascontiguousarray(x.T)
    onehot = (wi[None, :] == np.arange(L, dtype=wi.dtype)[:, None])
    import ml_dtypes
    mt = np.repeat(onehot, R, axis=0).astype(ml_dtypes.bfloat16)  # [L*R, T]

    in_maps = []
    for c in range(NCORES):
        h = c // 4
        lo = h * BDIM + (c % 4) * CPC
        gcols = slice(lo, lo + CPC)
        wt_c = np.ascontiguousarray(W[gcols, :].T)  # [D, CPC]
        ac_c = np.ascontiguousarray(
            A[:, :, h * R:(h + 1) * R].transpose(1, 0, 2).reshape(D, LR)
        )
        bc_c = np.ascontiguousarray(B[:, :, gcols].reshape(LR, CPC))
        in_maps.append({"xt": xt, "wt": wt_c, "ac": ac_c, "bc": bc_c, "mt": mt})
    return in_maps


def assemble_output(results):
    out = np.empty((T, 2 * BDIM), dtype=np.float32)
    for c in range(NCORES):
        h = c // 4
        lo = h * BDIM + (c % 4) * CPC
        out[:, lo:lo + CPC] = results[c]["out"].T
    return out


def kernel(x, W, A_buffer, B_buffer, weight_indices):
    from concourse.bass_utils import run_bass_kernel_spmd

    in_maps = make_in_maps(x, W, A_buffer, B_buffer, weight_indices)
    nc = get_program()
    res = run_bass_kernel_spmd(
        nc, in_maps, core_ids=list(range(NCORES)), trace=False
    )
    return assemble_output(res.results)


def _make_runner(nc, donate=True):
    """Build a jitted 8-core runner (mirrors bass2jax.run_bass_via_pjrt).
    With donate=False, inputs/zero-outs stay device-resident across calls,
    so repeated calls re-execute the NEFF without re-uploading data."""
    import jax
    import concourse.mybir as mybir
    from jax.sharding import Mesh, NamedSharding, PartitionSpec
    from jax.experimental.shard_map import shard_map
    from concourse.bass2jax import (
        _bass_exec_p,
        install_neuronx_cc_hook,
        partition_id_tensor,
    )

    install_neuronx_cc_hook()

    partition_name = (
        nc.partition_id_tensor.name if nc.partition_id_tensor else None
    )
    in_names, out_names, out_avals, zero_outs = [], [], [], []
    for alloc in nc.m.functions[0].allocations:
        if not isinstance(alloc, mybir.MemoryLocationSet):
            continue
        name = alloc.memorylocations[0].name
        if alloc.kind == "ExternalInput":
            if name != partition_name:
                in_names.append(name)
        elif alloc.kind == "ExternalOutput":
            out_names.append(name)
            shape = tuple(alloc.tensor_shape)
            dtype = mybir.dt.np(alloc.dtype)
            out_avals.append(jax.core.ShapedArray(shape, dtype))
            zero_outs.append(np.zeros(shape, dtype))
    n_params = len(in_names)
    n_outs = len(out_avals)
    all_names = list(in_names) + list(out_names)
    if partition_name is not None:
        all_names.append(partition_name)
    all_names = tuple(all_names)

    def _body(*args):
        operands = list(args)
        if partition_name is not None:
            operands.append(partition_id_tensor())
        outs = _bass_exec_p.bind(
            *operands,
            out_avals=tuple(out_avals),
            in_names=all_names,
            out_names=tuple(out_names),
            lowering_input_output_aliases=(),
            sim_require_finite=True,
            sim_require_nnan=True,
            nc=nc,
        )
        return tuple(outs)

    devices = jax.devices()[:NCORES]
    mesh = Mesh(np.asarray(devices), ("core",))
    in_specs = (PartitionSpec("core"),) * (n_params + n_outs)
    out_specs = (PartitionSpec("core"),) * n_outs
    sharded = jax.jit(
        shard_map(
            _body, mesh=mesh, in_specs=in_specs, out_specs=out_specs,
            check_rep=False,
        ),
        donate_argnums=(
            tuple(range(n_params, n_params + n_outs)) if donate else ()
        ),
        keep_unused=True,
    )

    sharding = NamedSharding(mesh, PartitionSpec("core"))

    def put(in_maps):
        import jax
        concat_in = [
            np.concatenate([in_maps[c][name] for c in range(NCORES)], axis=0)
            for name in in_names
        ]
        concat_zeros = [
            np.zeros((NCORES * z.shape[0], *z.shape[1:]), z.dtype)
            for z in zero_outs
        ]
        return [jax.device_put(a, sharding) for a in concat_in + concat_zeros]

    def unpack(out_arrs):
        return [
            {
                name: np.asarray(out_arrs[i]).reshape(
                    NCORES, *out_avals[i].shape
                )[c]
                for i, name in enumerate(out_names)
            }
            for c in range(NCORES)
        ]

    return sharded, put, unpack


def bench(x, W, A_buffer, B_buffer, weight_indices, iters=24):
    """Returns (output, per_exec_ns, info). Fires `iters` async executions
    with device-resident inputs and blocks at the end, so per-call dispatch
    overlaps execution; the amortized delta approximates HW exec time."""
    import time

    import jax

    in_maps = make_in_maps(x, W, A_buffer, B_buffer, weight_indices)
    nc = get_program()
    sharded, put, unpack = _make_runner(nc, donate=False)
    dev_args = put(in_maps)

    outs = jax.block_until_ready(sharded(*dev_args))  # compile + warm-up
    results = unpack(outs)

    def burst(k):
        t0 = time.monotonic()
        rs = [sharded(*dev_args) for _ in range(k)]
        jax.block_until_ready(rs)
        return time.monotonic() - t0

    burst(2)  # extra warm-up
    t_small = min(burst(2) for _ in range(3))
    t_big = min(burst(2 + iters) for _ in range(3))
    per_exec_ns = (t_big - t_small) / iters * 1e9
    info = {
        "t_small_s": t_small,
        "t_big_s": t_big,
        "iters": iters,
        "per_exec_ns": per_exec_ns,
    }
    return assemble_output(results), per_exec_ns, info


# revision 24
# speedup vs baseline: 1.0960x; 1.0960x over previous
# Trainium2 Bass kernel for MergedColumnParallelLinearWithTopping
# (base column-parallel GEMM + per-token LoRA "topping", Punica-style).
#
# Math per core c (of 8, column-parallel over the 2*BDIM output dim):
#   out_c = x @ Wc.T + ((x @ Ac) * Mc) @ Bc
# where Ac = concat_l A_buffer[l,:,half]  [D, L*R]
#       Bc = concat_l B_buffer[l,:,cols]  [L*R, CPC]
#       M[t, l*R+r] = (weight_indices[t] == l)   (host-precomputed one-hot)
# This turns the per-token gather into two dense GEMMs that accumulate in
# the same PSUM group as the base GEMM.  All matmuls run as float32r
# (full PE rate for moving free dim >= 256).
#
# Self-contained: hardcodes shapes, builds the Bass program, shards inputs,
# runs on cores 0-7 via run_bass_kernel_spmd, reassembles the full output.

import numpy as np

# Problem shapes (hardcoded per spec)
T, D = 2048, 2048
L, R = 16, 16
BDIM = 5632
NCORES = 8
CPC = 2 * BDIM // NCORES  # 1408 output cols per core
P = 128
KO = D // P               # 16 contraction chunks
TS = 512                  # token-slice (matmul moving free dim)
NT = T // TS              # 4
MCH = CPC // P            # 11 output-column chunks per core
LR = L * R                # 256 (one half's lora rows)
LRO = LR // P             # 2

_PROGRAM_CACHE = {}


def _build_program():
    import concourse.bacc as bacc
    import concourse.tile as tile
    from concourse import mybir

    f32 = mybir.dt.float32
    f32r = mybir.dt.float32r
    bf16 = mybir.dt.bfloat16

    nc = bacc.Bacc("TRN2", target_bir_lowering=False, debug=False)

    # All inputs arrive pre-packed on the host into SBUF layout, so every
    # DMA reads/writes long contiguous per-partition runs.
    xt_r = nc.dram_tensor("xt", [NT, P, KO, TS], f32r, kind="ExternalInput").ap()
    wt_r = nc.dram_tensor("wt", [MCH, P, KO, P], f32r, kind="ExternalInput").ap()
    ac_r = nc.dram_tensor("ac", [P, KO, LR], f32r, kind="ExternalInput").ap()
    bc_r = nc.dram_tensor("bc", [P, LRO, CPC], f32r, kind="ExternalInput").ap()
    mt_r = nc.dram_tensor("mt", [NT, P, LRO, TS], bf16, kind="ExternalInput").ap()
    out_r = nc.dram_tensor("out", [MCH, P, NT, TS], f32, kind="ExternalOutput").ap()

    with tile.TileContext(nc) as tc:
        with (
            tc.tile_pool(name="xres", bufs=NT) as xpool,
            tc.tile_pool(name="wpool", bufs=2) as wpool,
            tc.tile_pool(name="consts", bufs=1) as cpool,
            tc.tile_pool(name="mtp", bufs=NT) as mtpool,
            tc.tile_pool(name="outp", bufs=3) as outpool,
            tc.tile_pool(name="psout", bufs=4, space="PSUM") as psout,
            tc.tile_pool(name="psxa", bufs=2, space="PSUM") as psxa,
        ):
            # Split DMAs that feed matmul operands into k-groups: the fused
            # fp32r matmul's LDWEIGHTS has very few semaphore-wait slots, so
            # each matmul must depend on at most one small DMA.
            KG = 4  # k-chunks per sub-DMA

            # Constants resident in SBUF
            a_sb = cpool.tile([P, KO, LR], f32r, name="a_sb")
            for kg in range(0, KO, KG):
                nc.sync.dma_start(
                    a_sb[:, kg:kg + KG, :], ac_r[:, kg:kg + KG, :]
                )
            b_sb = cpool.tile([P, LRO, CPC], f32r, name="b_sb")
            for o in range(LRO):
                nc.sync.dma_start(b_sb[:, o, :], bc_r[:, o, :])
            # masked x@A activation, filled per token-slice below
            xam = cpool.tile([P, LRO, T], f32r, name="xam")

            # x fully resident, loaded as NT independent slices so deps are
            # per-slice (and per k-group within a slice)
            x_sb = []
            for t in range(NT):
                xs = xpool.tile([P, KO, TS], f32r, name=f"x{t}", tag="x")
                for kg in range(0, KO, KG):
                    nc.sync.dma_start(
                        xs[:, kg:kg + KG, :], xt_r[t, :, kg:kg + KG, :]
                    )
                x_sb.append(xs)

            def w_load(m):
                # single DMA per chunk: one queue, so slot-reuse WAW is one
                # semaphore and the guard absorbs the single data wait
                wtile = wpool.tile([P, KO, P], f32r, name=f"w{m}", tag="w")
                nc.sync.dma_start(wtile[:], wt_r[m])
                return wtile

            w_tiles = {0: w_load(0)}

            def base_group(m, wtile, t):
                # one [128, TS] output tile: 16 base matmuls + 2 lora matmuls
                # accumulating in the same PSUM bank
                ps = psout.tile([P, TS], f32, name=f"ps_{m}_{t}", tag="ps")
                for k in range(KO):
                    nc.tensor.matmul(
                        ps[:],
                        lhsT=wtile[:, k, :],
                        rhs=x_sb[t][:, k, :],
                        start=(k == 0),
                        stop=False,
                    )
                for k2 in range(LRO):
                    nc.tensor.matmul(
                        ps[:],
                        lhsT=b_sb[:, k2, m * P:(m + 1) * P],
                        rhs=xam[:, k2, t * TS:(t + 1) * TS],
                        start=False,
                        stop=(k2 == LRO - 1),
                    )
                o = outpool.tile([P, TS], f32, name=f"o_{m}_{t}", tag="o")
                nc.any.tensor_copy(out=o[:], in_=ps[:])
                nc.sync.dma_start(out_r[m, :, t, :], o[:])

            # Pass 1 over token-slices: compute masked XA, then first W chunk
            for t in range(NT):
                mt_sb = mtpool.tile([P, LRO, TS], bf16, name=f"mt{t}", tag="mt")
                for o in range(LRO):
                    nc.sync.dma_start(mt_sb[:, o, :], mt_r[t, :, o, :])
                for mp in range(LRO):
                    pxa = psxa.tile([P, TS], f32, name=f"pxa_{t}_{mp}", tag="pxa")
                    for k in range(KO):
                        nc.tensor.matmul(
                            pxa[:],
                            lhsT=a_sb[:, k, mp * P:(mp + 1) * P],
                            rhs=x_sb[t][:, k, :],
                            start=(k == 0),
                            stop=(k == KO - 1),
                        )
                    nc.vector.tensor_tensor(
                        xam[:, mp, t * TS:(t + 1) * TS],
                        pxa[:],
                        mt_sb[:, mp, :],
                        mybir.AluOpType.mult,
                    )
                if t == 0:
                    w_tiles[1] = w_load(1)
                base_group(0, w_tiles[0], t)

            # Remaining W chunks, x stays resident
            for m in range(1, MCH):
                if m + 1 < MCH and (m + 1) not in w_tiles:
                    w_tiles[m + 1] = w_load(m + 1)
                for t in range(NT):
                    base_group(m, w_tiles[m], t)

    nc.compile()
    return nc


def get_program():
    if "nc" not in _PROGRAM_CACHE:
        _PROGRAM_CACHE["nc"] = _build_program()
    return _PROGRAM_CACHE["nc"]


def make_in_maps(x, W, A_buffer, B_buffer, weight_indices):
    x = np.ascontiguousarray(np.asarray(x, dtype=np.float32))
    W = np.asarray(W, dtype=np.float32)
    A = np.asarray(A_buffer, dtype=np.float32)
    B = np.asarray(B_buffer, dtype=np.float32)
    wi = np.asarray(weight_indices).astype(np.int64)

    # pack to SBUF layout [.., P(partition), .., contiguous free dims]
    xt = np.ascontiguousarray(
        x.T.reshape(KO, P, NT, TS).transpose(2, 1, 0, 3)
    )  # [NT, P, KO, TS]
    onehot = (wi[None, :] == np.arange(L, dtype=wi.dtype)[:, None])
    import ml_dtypes
    mt = np.ascontiguousarray(
        np.repeat(onehot, R, axis=0)
        .reshape(LRO, P, NT, TS)
        .transpose(2, 1, 0, 3)
    ).astype(ml_dtypes.bfloat16)  # [NT, P, LRO, TS]

    in_maps = []
    for c in range(NCORES):
        h = c // 4
        lo = h * BDIM + (c % 4) * CPC
        gcols = slice(lo, lo + CPC)
        wt_c = np.ascontiguousarray(
            W[gcols, :].T.reshape(KO, P, MCH, P).transpose(2, 1, 0, 3)
        )  # [MCH, P, KO, P]
        ac_c = np.ascontiguousarray(
            A[:, :, h * R:(h + 1) * R]
            .transpose(1, 0, 2).reshape(KO, P, LR).transpose(1, 0, 2)
        )  # [P, KO, LR]
        bc_c = np.ascontiguousarray(
            B[:, :, gcols].reshape(LRO, P, CPC).transpose(1, 0, 2)
        )  # [P, LRO, CPC]
        in_maps.append({"xt": xt, "wt": wt_c, "ac": ac_c, "bc": bc_c, "mt": mt})
    return in_maps


def assemble_output(results):
    out = np.empty((T, 2 * BDIM), dtype=np.float32)
    for c in range(NCORES):
        h = c // 4
        lo = h * BDIM + (c % 4) * CPC
        # [MCH, P, NT, TS] -> [tok, col]
        piece = results[c]["out"].transpose(2, 3, 0, 1).reshape(T, CPC)
        out[:, lo:lo + CPC] = piece
    return out


def kernel(x, W, A_buffer, B_buffer, weight_indices):
    from concourse.bass_utils import run_bass_kernel_spmd

    in_maps = make_in_maps(x, W, A_buffer, B_buffer, weight_indices)
    nc = get_program()
    res = run_bass_kernel_spmd(
        nc, in_maps, core_ids=list(range(NCORES)), trace=False
    )
    return assemble_output(res.results)


def _make_runner(nc, donate=True):
    """Build a jitted 8-core runner (mirrors bass2jax.run_bass_via_pjrt).
    With donate=False, inputs/zero-outs stay device-resident across calls,
    so repeated calls re-execute the NEFF without re-uploading data."""
    import jax
    import concourse.mybir as mybir
    from jax.sharding import Mesh, NamedSharding, PartitionSpec
    from jax.experimental.shard_map import shard_map
    from concourse.bass2jax import (
        _bass_exec_p,
        install_neuronx_cc_hook,
        partition_id_tensor,
    )

    install_neuronx_cc_hook()

    partition_name = (
        nc.partition_id_tensor.name if nc.partition_id_tensor else None
    )
    in_names, out_names, out_avals, zero_outs = [], [], [], []
    for alloc in nc.m.functions[0].allocations:
        if not isinstance(alloc, mybir.MemoryLocationSet):
            continue
        name = alloc.memorylocations[0].name
        if alloc.kind == "ExternalInput":
            if name != partition_name:
                in_names.append(name)
        elif alloc.kind == "ExternalOutput":
            out_names.append(name)
            shape = tuple(alloc.tensor_shape)
            dtype = mybir.dt.np(alloc.dtype)
            out_avals.append(jax.core.ShapedArray(shape, dtype))
            zero_outs.append(np.zeros(shape, dtype))
    n_params = len(in_names)
    n_outs = len(out_avals)
    all_names = list(in_names) + list(out_names)
    if partition_name is not None:
        all_names.append(partition_name)
    all_names = tuple(all_names)

    def _body(*args):
        operands = list(args)
        if partition_name is not None:
            operands.append(partition_id_tensor())
        outs = _bass_exec_p.bind(
            *operands,
            out_avals=tuple(out_avals),
            in_names=all_names,
            out_names=tuple(out_names),
            lowering_input_output_aliases=(),
            sim_require_finite=True,
            sim_require_nnan=True,
            nc=nc,
        )
        return tuple(outs)

    devices = jax.devices()[:NCORES]
    mesh = Mesh(np.asarray(devices), ("core",))
    in_specs = (PartitionSpec("core"),) * (n_params + n_outs)
    out_specs = (PartitionSpec("core"),) * n_outs
    sharded = jax.jit(
        shard_map(
            _body, mesh=mesh, in_specs=in_specs, out_specs=out_specs,
            check_rep=False,
        ),
        donate_argnums=(
            tuple(range(n_params, n_params + n_outs)) if donate else ()
        ),
        keep_unused=True,
    )

    sharding = NamedSharding(mesh, PartitionSpec("core"))

    def put(in_maps):
        import jax
        concat_in = [
            np.concatenate([in_maps[c][name] for c in range(NCORES)], axis=0)
            for name in in_names
        ]
        concat_zeros = [
            np.zeros((NCORES * z.shape[0], *z.shape[1:]), z.dtype)
            for z in zero_outs
        ]
        return [jax.device_put(a, sharding) for a in concat_in + concat_zeros]

    def unpack(out_arrs):
        return [
            {
                name: np.asarray(out_arrs[i]).reshape(
                    NCORES, *out_avals[i].shape
                )[c]
                for i, name in enumerate(out_names)
            }
            for c in range(NCORES)
        ]

    return sharded, put, unpack


def bench(x, W, A_buffer, B_buffer, weight_indices, iters=24):
    """Returns (output, per_exec_ns, info). Fires `iters` async executions
    with device-resident inputs and blocks at the end, so per-call dispatch
    overlaps execution; the amortized delta approximates HW exec time."""
    import time

    import jax

    in_maps = make_in_maps(x, W, A_buffer, B_buffer, weight_indices)
    nc = get_program()
    sharded, put, unpack = _make_runner(nc, donate=False)
    dev_args = put(in_maps)

    outs = jax.block_until_ready(sharded(*dev_args))  # compile + warm-up
    results = unpack(outs)

    def burst(k):
        t0 = time.monotonic()
        rs = [sharded(*dev_args) for _ in range(k)]
        jax.block_until_ready(rs)
        return time.monotonic() - t0

    burst(2)  # extra warm-up
    t_small = min(burst(2) for _ in range(3))
    t_big = min(burst(2 + iters) for _ in range(3))
    per_exec_ns = (t_big - t_small) / iters * 1e9
    info = {
        "t_small_s": t_small,
        "t_big_s": t_big,
        "iters": iters,
        "per_exec_ns": per_exec_ns,
    }
    return assemble_output(results), per_exec_ns, info
